# revision 8
# baseline (speedup 1.0000x reference)
"""Approximate rank pooling (segment-reduce) on 8 TRN2 NeuronCores.

Strategy: the per-frame weight w[t] depends only on vidids (tiny), so it is
computed on the host exactly as the reference does. The heavy part -- the
weighted segment sum over x [2048, 3*128*128] -- runs on device as a matmul:
each core c gets an equal slice of 256 frames, x_c [256, 49152], plus a
host-built weight matrix W_c [256, NV] whose row i has w[global_t] at column
(vidids[global_t] - v_lo_c) and zeros elsewhere.  The core computes
out_c = W_c^T @ x_c with TensorEngine accumulating over the two 128-frame
K-chunks in PSUM.  The host then scatters/adds the per-core partial outputs
into the full [64, 3, 128, 128] result (a video straddling a core boundary
simply gets contributions from both cores).
"""

import numpy as np

T, C, H, W = 2048, 3, 128, 128
D = C * H * W              # 49152
NCORES = 8
TL = T // NCORES           # 256 frames per core
KP = 128                   # K chunk = SBUF partition count
NK = TL // KP              # 2 K-chunks
CHUNK = 8192               # columns of x per load (32 KiB contiguous rows)
NJ = D // CHUNK            # 6
OCHUNK = 4096              # columns per output store tile
NO = CHUNK // OCHUNK       # 2
SUB = 512                  # matmul moving-dim limit (fp32) = one PSUM bank
NSUB = OCHUNK // SUB       # 8

MM_DTYPE = "float32r"      # "float32r" (1 cyc/row) or "float32" (4 cyc/row)


def _frame_weights(vid: np.ndarray, nvids: int) -> np.ndarray:
    """Replicates the reference weight math in numpy (float32)."""
    T_ = vid.shape[0]
    counts = np.bincount(vid, minlength=nvids).astype(np.int64)
    starts = np.cumsum(counts) - counts
    N = counts[vid]                                    # [T] segment size
    t = np.arange(T_, dtype=np.int64) - starts[vid] + 1  # [T] 1-based rank
    Hh = np.zeros(T_ + 1, dtype=np.float32)
    Hh[1:] = np.cumsum(
        (1.0 / np.arange(1, T_ + 1, dtype=np.float32)).astype(np.float32),
        dtype=np.float32,
    )
    poly = (N * (N + 1) - t * (t - 1) - N * (N - t + 1)).astype(np.float32)
    w = poly - (Hh[N] - Hh[t - 1])
    return np.where(N == 1, np.float32(1.0), w).astype(np.float32)


def _build_nc(nv: int, mm_dtype: str):
    import concourse.bacc as bacc
    import concourse.tile as tile
    from concourse import mybir

    dt = getattr(mybir.dt, mm_dtype)
    f32 = mybir.dt.float32

    nc = bacc.Bacc("TRN2", target_bir_lowering=False, debug=False)
    x = nc.dram_tensor("x", [TL, D], dt, kind="ExternalInput").ap()
    wt = nc.dram_tensor("wt", [TL, nv], dt, kind="ExternalInput").ap()
    out = nc.dram_tensor("out", [nv, D], f32, kind="ExternalOutput").ap()

    with tile.TileContext(nc) as tc:
        with (
            tc.tile_pool(name="wpool", bufs=1) as wpool,
            tc.tile_pool(name="xpool", bufs=4) as xpool,
            tc.tile_pool(name="opool", bufs=2) as opool,
            tc.tile_pool(name="psum", bufs=8, space="PSUM") as ppool,
        ):
            wtiles = []
            for k in range(NK):
                wtile = wpool.tile([KP, nv], dt, tag=f"w{k}")
                nc.gpsimd.dma_start(wtile[:], wt[k * KP:(k + 1) * KP, :])
                wtiles.append(wtile)

            for j in range(NJ):
                xts = []
                for k in range(NK):
                    xt = xpool.tile([KP, CHUNK], dt)
                    nc.sync.dma_start(
                        xt[:],
                        x[k * KP:(k + 1) * KP, j * CHUNK:(j + 1) * CHUNK],
                    )
                    xts.append(xt)

                for o in range(NO):
                    pts = [
                        ppool.tile([nv, SUB], f32, name="pt", tag="pt")
                        for _ in range(NSUB)
                    ]
                    for k in range(NK):
                        for s in range(NSUB):
                            col = o * OCHUNK + s * SUB
                            nc.tensor.matmul(
                                pts[s][:],
                                wtiles[k][:],
                                xts[k][:, col:col + SUB],
                                start=(k == 0),
                                stop=(k == NK - 1),
                            )

                    ot = opool.tile([nv, OCHUNK], f32)
                    for s in range(NSUB):
                        nc.vector.tensor_copy(
                            ot[:, s * SUB:(s + 1) * SUB], pts[s][:]
                        )
                    ocol = j * CHUNK + o * OCHUNK
                    nc.scalar.dma_start(out[:, ocol:ocol + OCHUNK], ot[:])

    nc.compile()
    return nc


def _run(x, vidids, nvids, trace=False, trace_cores=None):
    from concourse.bass_utils import run_bass_kernel_spmd

    x = np.ascontiguousarray(np.asarray(x, dtype=np.float32))
    vid = np.asarray(vidids).astype(np.int64).ravel()
    nv_total = int(nvids)
    assert x.shape == (T, C, H, W) and vid.shape == (T,)

    w = _frame_weights(vid, nv_total)
    xf = x.reshape(T, D)

    v_lo, nv_local = [], []
    for c in range(NCORES):
        lo, hi = c * TL, (c + 1) * TL
        v_lo.append(int(vid[lo]))
        nv_local.append(int(vid[hi - 1]) - int(vid[lo]) + 1)
    NV = max(nv_local)

    in_maps = []
    rows = np.arange(TL)
    for c in range(NCORES):
        lo = c * TL
        Wc = np.zeros((TL, NV), dtype=np.float32)
        Wc[rows, vid[lo:lo + TL] - v_lo[c]] = w[lo:lo + TL]
        in_maps.append({"x": xf[lo:lo + TL], "wt": Wc})

    nc = _build_nc(NV, MM_DTYPE)
    res = run_bass_kernel_spmd(
        nc, in_maps, list(range(NCORES)), trace=trace, trace_cores=trace_cores
    )

    outf = np.zeros((nv_total, D), dtype=np.float32)
    for c in range(NCORES):
        part = res.results[c]["out"]
        n = min(NV, nv_total - v_lo[c])
        outf[v_lo[c]:v_lo[c] + n] += part[:n]
    return outf.reshape(nv_total, C, H, W), res


def kernel(x, vidids, nvids):
    out, _ = _run(x, vidids, nvids)
    return out


# revision 11
# speedup vs baseline: 1.2393x; 1.2393x over previous
"""Approximate rank pooling (segment-reduce) on 8 TRN2 NeuronCores.

Strategy: the per-frame weight w[t] depends only on vidids (tiny), so it is
computed on the host exactly as the reference does. The heavy part -- the
weighted segment sum over x [2048, 3*128*128] -- runs on device as a matmul:
each core c gets an equal slice of 256 frames, x_c [256, 49152], plus a
host-built weight matrix W_c [256, NV] whose row i has w[global_t] at column
(vidids[global_t] - v_lo_c) and zeros elsewhere.  The core computes
out_c = W_c^T @ x_c with TensorEngine accumulating over the two 128-frame
K-chunks in PSUM.  The host then scatters/adds the per-core partial outputs
into the full [64, 3, 128, 128] result (a video straddling a core boundary
simply gets contributions from both cores).
"""

import numpy as np

T, C, H, W = 2048, 3, 128, 128
D = C * H * W              # 49152
NCORES = 8
TL = T // NCORES           # 256 frames per core
KP = 128                   # K chunk = SBUF partition count
NK = TL // KP              # 2 K-chunks
CHUNK = 4096               # columns of x per load (16 KiB contiguous rows --
                           # the per-SDMA-queue descriptor sweet spot)
NJ = D // CHUNK            # 12
SUB = 512                  # matmul moving-dim limit (fp32) = one PSUM bank
NSUB = CHUNK // SUB        # 8

MM_DTYPE = "float32r"      # "float32r" (1 cyc/row) or "float32" (4 cyc/row)


def _frame_weights(vid: np.ndarray, nvids: int) -> np.ndarray:
    """Replicates the reference weight math in numpy (float32)."""
    T_ = vid.shape[0]
    counts = np.bincount(vid, minlength=nvids).astype(np.int64)
    starts = np.cumsum(counts) - counts
    N = counts[vid]                                    # [T] segment size
    t = np.arange(T_, dtype=np.int64) - starts[vid] + 1  # [T] 1-based rank
    Hh = np.zeros(T_ + 1, dtype=np.float32)
    Hh[1:] = np.cumsum(
        (1.0 / np.arange(1, T_ + 1, dtype=np.float32)).astype(np.float32),
        dtype=np.float32,
    )
    poly = (N * (N + 1) - t * (t - 1) - N * (N - t + 1)).astype(np.float32)
    w = poly - (Hh[N] - Hh[t - 1])
    return np.where(N == 1, np.float32(1.0), w).astype(np.float32)


def _build_nc(nv: int, mm_dtype: str):
    import concourse.bacc as bacc
    import concourse.tile as tile
    from concourse import mybir

    dt = getattr(mybir.dt, mm_dtype)
    f32 = mybir.dt.float32

    nc = bacc.Bacc("TRN2", target_bir_lowering=False, debug=False)
    x = nc.dram_tensor("x", [TL, D], dt, kind="ExternalInput").ap()
    wt = nc.dram_tensor("wt", [TL, nv], dt, kind="ExternalInput").ap()
    out = nc.dram_tensor("out", [nv, D], f32, kind="ExternalOutput").ap()

    with tile.TileContext(nc) as tc:
        with (
            tc.tile_pool(name="wpool", bufs=1) as wpool,
            tc.tile_pool(name="xpool", bufs=6) as xpool,
            tc.tile_pool(name="opool", bufs=2) as opool,
            tc.tile_pool(name="psum", bufs=8, space="PSUM") as ppool,
        ):
            wtiles = []
            for k in range(NK):
                wtile = wpool.tile([KP, nv], dt, tag=f"w{k}")
                nc.gpsimd.dma_start(wtile[:], wt[k * KP:(k + 1) * KP, :])
                wtiles.append(wtile)

            for j in range(NJ):
                xts = []
                for k in range(NK):
                    xt = xpool.tile([KP, CHUNK], dt)
                    nc.sync.dma_start(
                        xt[:],
                        x[k * KP:(k + 1) * KP, j * CHUNK:(j + 1) * CHUNK],
                    )
                    xts.append(xt)

                pts = [
                    ppool.tile([nv, SUB], f32, name="pt", tag="pt")
                    for _ in range(NSUB)
                ]
                for k in range(NK):
                    for s in range(NSUB):
                        nc.tensor.matmul(
                            pts[s][:],
                            wtiles[k][:],
                            xts[k][:, s * SUB:(s + 1) * SUB],
                            start=(k == 0),
                            stop=(k == NK - 1),
                        )

                ot = opool.tile([nv, CHUNK], f32)
                for s in range(NSUB):
                    nc.any.tensor_copy(ot[:, s * SUB:(s + 1) * SUB], pts[s][:])
                nc.gpsimd.dma_start(out[:, j * CHUNK:(j + 1) * CHUNK], ot[:])

    nc.compile()
    return nc


def _run(x, vidids, nvids, trace=False, trace_cores=None):
    from concourse.bass_utils import run_bass_kernel_spmd

    x = np.ascontiguousarray(np.asarray(x, dtype=np.float32))
    vid = np.asarray(vidids).astype(np.int64).ravel()
    nv_total = int(nvids)
    assert x.shape == (T, C, H, W) and vid.shape == (T,)

    w = _frame_weights(vid, nv_total)
    xf = x.reshape(T, D)

    v_lo, nv_local = [], []
    for c in range(NCORES):
        lo, hi = c * TL, (c + 1) * TL
        v_lo.append(int(vid[lo]))
        nv_local.append(int(vid[hi - 1]) - int(vid[lo]) + 1)
    NV = max(nv_local)

    in_maps = []
    rows = np.arange(TL)
    for c in range(NCORES):
        lo = c * TL
        Wc = np.zeros((TL, NV), dtype=np.float32)
        Wc[rows, vid[lo:lo + TL] - v_lo[c]] = w[lo:lo + TL]
        in_maps.append({"x": xf[lo:lo + TL], "wt": Wc})

    nc = _build_nc(NV, MM_DTYPE)
    res = run_bass_kernel_spmd(
        nc, in_maps, list(range(NCORES)), trace=trace, trace_cores=trace_cores
    )

    outf = np.zeros((nv_total, D), dtype=np.float32)
    for c in range(NCORES):
        part = res.results[c]["out"]
        n = min(NV, nv_total - v_lo[c])
        outf[v_lo[c]:v_lo[c] + n] += part[:n]
    return outf.reshape(nv_total, C, H, W), res


def kernel(x, vidids, nvids):
    out, _ = _run(x, vidids, nvids)
    return out
